# revision 2
# baseline (speedup 1.0000x reference)
"""Trainium2 Bass kernel for nn_MOAB_46273977647401 (v5).

Network (reference):
  x1 (256,256), x3 (256,) -> 4 outer sigmoid maps (256,257,257)
  -> 1x1 conv combine (4ch) + eval BN + leaky(0.1) -> (256, 66049)
  -> FC (66049 -> 512) + relu -> FC (512 -> 4)

Sharding: 8 cores = 2 batch shards (BC=128) x 4 contraction shards
(i-quarters of 64 rows).  Each core computes z for its i-quarter in
[(parity,i_local)-partitions, (jpair,b)-free] layout so a K=128 matmul
contracts 64 i-rows x 2 j-columns at once against host-relaid fc_w
slabs with the full H=512 on the free axis.  Each core emits its f32
partial FC1 accumulator [128b, 512h]; the host sums the 4 partials per
batch shard and runs the tiny relu+FC2 tail (~1 MFLOP).

v4 vs v2: hot-path consts DMA first on the sync queue while bulk
setup DMAs ride the DVE queue, the j=0 y-values (batch independent)
come precomputed from the host, and the strip is emitted before the
last chunk so its matmuls don't tail-gate the PSUM drain.
"""

import numpy as np

import concourse.bass as bass
import concourse.tile as tile
from concourse import bacc, mybir
from concourse.bass_utils import run_bass_kernel_spmd

F32 = mybir.dt.float32
BF16 = mybir.dt.bfloat16
AL = mybir.AluOpType

B, N, H, C = 256, 256, 512, 4
NP = 257                  # N+1
P_B, P_K = 2, 4           # batch shards x contraction (i) shards
BC = B // P_B             # 128 batch rows per core
IQ = 64                   # i rows per core (i-quarter)
JC = 8                    # j-PAIRS per chunk (16 j values)
CH = JC * BC              # 1024 free elems per chunk
NCHUNK = 128 // JC        # 16 chunks (128 j-pairs, j in [1,257))
EPS = 1e-10
BN_EPS = 1e-5
LEAKY = 0.1

W_DTYPE = BF16
W_NP = np.dtype("bfloat16")


def build_program():
    nc = bacc.Bacc("TRN2", target_bir_lowering=False, debug=False, num_devices=8)

    d_a0T = nc.dram_tensor("a0T", [NP, BC], F32, kind="ExternalInput").ap()
    d_a1T = nc.dram_tensor("a1T", [NP, BC], F32, kind="ExternalInput").ap()
    d_aflat = nc.dram_tensor("aflat", [2 * NCHUNK, CH], BF16,
                             kind="ExternalInput").ap()
    d_b0 = nc.dram_tensor("b0", [128, 1], F32, kind="ExternalInput").ap()
    d_b1 = nc.dram_tensor("b1", [128, 1], F32, kind="ExternalInput").ap()
    d_cv = nc.dram_tensor("cv", [128, 1], F32, kind="ExternalInput").ap()
    d_sv = nc.dram_tensor("sv", [128, 6], F32, kind="ExternalInput").ap()
    d_ind = nc.dram_tensor("ind", [2, 128], BF16, kind="ExternalInput").ap()
    d_yj0 = nc.dram_tensor("yj0", [IQ, 1], F32, kind="ExternalInput").ap()
    d_w3 = nc.dram_tensor("w3", [128, NCHUNK, JC * H], W_DTYPE,
                          kind="ExternalInput").ap()
    d_wcol0 = nc.dram_tensor("wcol0", [IQ, H], W_DTYPE, kind="ExternalInput").ap()
    d_wstrip = nc.dram_tensor("wstrip", [NP, H], W_DTYPE, kind="ExternalInput").ap()
    d_out = nc.dram_tensor("out", [BC, H], F32, kind="ExternalOutput").ap()

    with tile.TileContext(nc) as tc:
        with (
            tc.tile_pool(name="const", bufs=1) as cpool,
            tc.tile_pool(name="setup", bufs=1) as spool,
            tc.tile_pool(name="stage", bufs=3) as stpool,
            tc.tile_pool(name="w0", bufs=3) as wpool,
            tc.tile_pool(name="z", bufs=3) as zpool,
            tc.tile_pool(name="comb", bufs=3) as combpool,
            tc.tile_pool(name="ypool", bufs=3) as ypool,
            tc.tile_pool(name="fin", bufs=1) as finpool,
            tc.tile_pool(name="psA", bufs=2, space="PSUM") as psA,
            tc.tile_pool(name="psR", bufs=1, space="PSUM") as psR,
            tc.tile_pool(name="psO", bufs=1, space="PSUM") as psO,
        ):
            SIG = mybir.ActivationFunctionType.Sigmoid

            # rf16 prep rides at the very front: stg_r(0)'s transfer must
            # enter the (FIFO) DMA-engine line before the first W slab
            af16 = spool.tile([2 * NCHUNK, CH], BF16, tag="af16")
            nc.sync.dma_start(af16[:, :], d_aflat[:, :])
            rf16 = cpool.tile([2 * NCHUNK, CH], BF16, tag="rf16")
            rtmp16 = spool.tile([2 * NCHUNK, CH], F32, tag="rtmp16")
            nc.vector.tensor_scalar_add(rtmp16[:, :], af16[:, :], EPS)
            with nc.allow_low_precision(
                reason="r feeds sigmoid whose input is bf16-rounded anyway"
            ):
                nc.vector.reciprocal(rf16[:, :], rtmp16[:, :])

            ind = cpool.tile([2, 128], BF16, tag="ind")
            nc.sync.dma_start(ind[:, :], d_ind[:, :])
            b0t = cpool.tile([128, 1], F32, tag="b0t")
            b1t = cpool.tile([128, 1], F32, tag="b1t")
            nc.sync.dma_start(b0t[:, :], d_b0[:, :])
            nc.sync.dma_start(b1t[:, :], d_b1[:, :])
            cv = cpool.tile([128, 1], F32, tag="cv")
            sv = cpool.tile([128, 6], F32, tag="sv")
            nc.sync.dma_start(sv[:, :], d_sv[:, :])

            # ---------------- main accumulation ----------------
            psum_out = psO.tile([BC, H], F32, tag="acc")
            mm_started = [False]

            def acc_mm(lhsT, rhs, stop=False):
                nc.tensor.matmul(
                    psum_out[:, :],
                    lhsT,
                    rhs,
                    start=not mm_started[0],
                    stop=stop,
                    skip_group_check=True,
                )
                mm_started[0] = True

            a0 = [cpool.tile([128, BC], F32, tag=f"a0_{k}", name=f"a0_{k}") for k in range(3)]
            a1 = [cpool.tile([128, BC], F32, tag=f"a1_{k}", name=f"a1_{k}") for k in range(3)]
            rt = [cpool.tile([128, BC], F32, tag=f"r_{k}", name=f"r_{k}") for k in range(3)]

            def emit_strip_dmas():
                # inputs for the i=256 strip; emitted mid-loop so they queue
                # behind the first W slabs instead of ahead of them (they
                # wait on nothing, so no head-of-line risk)
                nc.sync.dma_start(cv[:, :], d_cv[:, :])
                nc.sync.dma_start(a0[0][:, :], d_a0T[0:128, :])
                nc.sync.dma_start(a0[1][:, :], d_a0T[128:256, :])
                nc.sync.dma_start(a0[2][0:1, :], d_a0T[256:257, :])
                nc.sync.dma_start(a1[0][:, :], d_a1T[0:128, :])
                nc.sync.dma_start(a1[1][:, :], d_a1T[128:256, :])
                nc.sync.dma_start(a1[2][0:1, :], d_a1T[256:257, :])

            def emit_strip_recips():
                for k, npart in ((0, 128), (1, 128), (2, 1)):
                    tmp = spool.tile([128, BC], F32, tag=f"rtmp_{k}")
                    nc.vector.tensor_scalar_add(
                        tmp[0:npart, :], a1[k][0:npart, :], EPS
                    )
                    nc.vector.reciprocal(rt[k][0:npart, :], tmp[0:npart, :])

            def emit_j0():
                # j=0 column: y values are batch independent, precomputed on
                # the host in f32; just broadcast and accumulate
                yl0 = spool.tile([128, 1], F32, tag="yl0")
                nc.sync.dma_start(yl0[0:IQ, :], d_yj0[:, :])
                yj0 = spool.tile([128, BC], W_DTYPE, tag="yj0")
                nc.vector.tensor_copy(yj0[0:IQ, :],
                                      yl0[0:IQ, 0:1].broadcast_to([IQ, BC]))
                wj0 = spool.tile([128, H], W_DTYPE, tag="wj0")
                nc.sync.dma_start(wj0[0:IQ, :], d_wcol0[:, :])
                acc_mm(yj0[0:IQ, :], wj0[0:IQ, :])

            def emit_strip():
                # i=256 strip (j in [0,257)); real weights only on the kq==3
                # core (zeros elsewhere)
                for jt, (jof, jsz) in enumerate(((0, 128), (128, 128), (256, 1))):
                    za = spool.tile([128, BC], F32, tag=f"sza_{jt}")
                    nc.scalar.activation(za[0:jsz, :], a0[jt][0:jsz, :], SIG,
                                         bias=cv[0:jsz, :], scale=1.0)
                    zs = spool.tile([128, BC], F32, tag=f"szs_{jt}")
                    nc.scalar.activation(zs[0:jsz, :], a0[jt][0:jsz, :], SIG,
                                         bias=cv[0:jsz, :], scale=-1.0)
                    zp = spool.tile([128, BC], F32, tag=f"szp_{jt}")
                    nc.scalar.activation(zp[0:jsz, :], a1[jt][0:jsz, :], SIG,
                                         bias=0.0, scale=cv[0:jsz, :])
                    zd = spool.tile([128, BC], F32, tag=f"szd_{jt}")
                    nc.scalar.activation(zd[0:jsz, :], rt[jt][0:jsz, :], SIG,
                                         bias=0.0, scale=cv[0:jsz, :])
                    t1 = spool.tile([128, BC], F32, tag=f"st1_{jt}")
                    nc.vector.tensor_scalar(t1[0:jsz, :], za[0:jsz, :],
                                            sv[0:jsz, 0:1], sv[0:jsz, 4:5],
                                            AL.mult, AL.add)
                    nc.vector.scalar_tensor_tensor(t1[0:jsz, :], zs[0:jsz, :],
                                                   sv[0:jsz, 1:2], t1[0:jsz, :],
                                                   AL.mult, AL.add)
                    nc.vector.scalar_tensor_tensor(t1[0:jsz, :], zp[0:jsz, :],
                                                   sv[0:jsz, 2:3], t1[0:jsz, :],
                                                   AL.mult, AL.add)
                    nc.vector.scalar_tensor_tensor(t1[0:jsz, :], zd[0:jsz, :],
                                                   sv[0:jsz, 3:4], t1[0:jsz, :],
                                                   AL.mult, AL.add)
                    yls = spool.tile([128, BC], W_DTYPE, tag=f"syl_{jt}")
                    nc.vector.scalar_tensor_tensor(yls[0:jsz, :], t1[0:jsz, :],
                                                   LEAKY, t1[0:jsz, :],
                                                   AL.mult, AL.max)
                    ws = spool.tile([128, H], W_DTYPE, tag=f"sws_{jt}")
                    nc.sync.dma_start(ws[0:jsz, :], d_wstrip[jof : jof + jsz, :])
                    acc_mm(yls[0:jsz, :], ws[0:jsz, :])

            for c in range(NCHUNK):
                # stage a/r rows for this chunk: [2, CH] (row 0: even j of
                # each pair, row 1: odd j).  stg_a goes via the Activation
                # DMA queue so chunk 0 doesn't wait for setup DMAs.
                stg_a = stpool.tile([2, CH], BF16, tag="stg_a")
                nc.sync.dma_start(stg_a[:, :], d_aflat[2 * c : 2 * c + 2, :])
                stg_r = stpool.tile([2, CH], BF16, tag="stg_r")
                nc.gpsimd.dma_start(stg_r[:, :], rf16[2 * c : 2 * c + 2, :])

                # indicator matmul: partitions 0-63 get row 0, 64-127 row 1
                arep = psA.tile([128, CH], F32, tag="arep")
                rrep = psR.tile([128, CH], F32, tag="rrep")
                for half in range(2):
                    sl = slice(half * 512, (half + 1) * 512)
                    nc.tensor.matmul(
                        arep[:, sl], ind[:, :], stg_a[:, sl],
                        start=True, stop=True, skip_group_check=True,
                    )
                    nc.tensor.matmul(
                        rrep[:, sl], ind[:, :], stg_r[:, sl],
                        start=True, stop=True, skip_group_check=True,
                    )

                za = zpool.tile([128, CH], BF16, tag="za")
                nc.scalar.activation(za[:, :], arep[:, :], SIG,
                                     bias=b0t[:, :], scale=1.0)
                zs = zpool.tile([128, CH], BF16, tag="zs")
                nc.scalar.activation(zs[:, :], arep[:, :], SIG,
                                     bias=b0t[:, :], scale=-1.0)
                zp = zpool.tile([128, CH], BF16, tag="zp")
                nc.scalar.activation(zp[:, :], arep[:, :], SIG,
                                     bias=0.0, scale=b1t[:, :])
                zd = zpool.tile([128, CH], BF16, tag="zd")
                nc.scalar.activation(zd[:, :], rrep[:, :], SIG,
                                     bias=0.0, scale=b1t[:, :])

                # W slab for this chunk: [128, JC*H]
                w = wpool.tile([128, JC * H], W_DTYPE, tag="wsl")
                nc.sync.dma_start(w[:, :], d_w3[:, c, :])

                # y = s0*za + s1*zs + s2*zp + s3*zd + off ; leaky
                ta = combpool.tile([128, CH], BF16, tag="ta")
                nc.vector.tensor_scalar(ta[:, :], za[:, :],
                                        sv[:, 0:1], sv[:, 4:5],
                                        AL.mult, AL.add)
                tb = combpool.tile([128, CH], BF16, tag="tb")
                nc.vector.tensor_scalar(tb[:, :], zs[:, :],
                                        sv[:, 1:2], None, AL.mult)
                tc2 = combpool.tile([128, CH], BF16, tag="tc2")
                nc.vector.tensor_scalar(tc2[:, :], zp[:, :],
                                        sv[:, 2:3], None, AL.mult)
                td = combpool.tile([128, CH], BF16, tag="td")
                nc.vector.tensor_scalar(td[:, :], zd[:, :],
                                        sv[:, 3:4], None, AL.mult)
                u1 = combpool.tile([128, CH], BF16, tag="u1")
                nc.vector.tensor_add(u1[:, :], ta[:, :], tb[:, :])
                u2 = combpool.tile([128, CH], BF16, tag="u2")
                nc.vector.tensor_add(u2[:, :], tc2[:, :], td[:, :])
                y1 = combpool.tile([128, CH], BF16, tag="y1")
                nc.vector.tensor_add(y1[:, :], u1[:, :], u2[:, :])
                lk = combpool.tile([128, CH], BF16, tag="lk")
                nc.vector.tensor_scalar(lk[:, :], y1[:, :],
                                        LEAKY, None, AL.mult)
                yl = ypool.tile([128, CH], W_DTYPE, tag="yl")
                nc.vector.tensor_tensor(yl[:, :], y1[:, :], lk[:, :],
                                        AL.max)

                for t in range(JC):
                    acc_mm(
                        yl[:, t * BC : (t + 1) * BC],
                        w[:, t * H : (t + 1) * H],
                        stop=(c == NCHUNK - 1 and t == JC - 1),
                    )

                if c == 2:
                    emit_j0()
                elif c == 3:
                    emit_strip_dmas()
                elif c == 5:
                    emit_strip_recips()
                elif c == NCHUNK - 2:
                    emit_strip()

            # ---------------- tail: emit f32 partial [BC, H] ----------------
            y2 = finpool.tile([BC, H], F32, tag="y2")
            nc.scalar.copy(y2[:, :], psum_out[:, :])
            nc.sync.dma_start(d_out[:, :], y2[:, :])

    nc.finalize()
    return nc


_CACHED_NC = None


def _get_program():
    global _CACHED_NC
    if _CACHED_NC is None:
        _CACHED_NC = build_program()
    return _CACHED_NC


def make_in_maps(x1, x3, conv_w, conv_b, bn_gamma, bn_beta, bn_mean, bn_var,
                 fc_w, fc_b, out_w, out_b):
    x1 = np.asarray(x1, np.float32)
    x3 = np.asarray(x3, np.float32)
    fc_w = np.asarray(fc_w, np.float32)

    g = float(np.asarray(bn_gamma).reshape(-1)[0]) / float(
        np.sqrt(np.asarray(bn_var).reshape(-1)[0] + BN_EPS))
    s = np.asarray(conv_w, np.float32).reshape(-1) * g
    off = (float(np.asarray(conv_b).reshape(-1)[0])
           - float(np.asarray(bn_mean).reshape(-1)[0])) * g \
        + float(np.asarray(bn_beta).reshape(-1)[0])

    sv = np.zeros((128, 6), np.float32)
    sv[:, 0], sv[:, 1], sv[:, 2], sv[:, 3] = s[0], s[1], s[2], s[3]
    sv[:, 4] = off
    sv[:, 5] = s[0] + s[1]
    cv = np.full((128, 1), x3[-1], np.float32)

    b0 = np.concatenate([[0.0], x3]).astype(np.float32)  # (257,)
    b1 = np.concatenate([[1.0], x3]).astype(np.float32)

    # j=0 column y values (batch independent), exact f32 on host:
    #   y0[i] = leaky((s0+s1)*sig(b0) + s2*sig(b1) + s3*sig(b1/(1+eps)) + off)
    def sigf(v):
        return 1.0 / (1.0 + np.exp(-v.astype(np.float64)))
    y0 = ((s[0] + s[1]) * sigf(b0) + s[2] * sigf(b1)
          + s[3] * sigf(b1 / (1.0 + EPS)) + off)
    y0 = np.where(y0 >= 0, y0, LEAKY * y0).astype(np.float32)

    ind = np.zeros((2, 128), np.float32)
    ind[0, :64] = 1.0
    ind[1, 64:] = 1.0
    ind = ind.astype(W_NP)

    # fc_w (H, 66049) with k = i*257+j  ->  W3 [i, j, h]
    w3 = np.ascontiguousarray(fc_w.reshape(H, NP, NP).transpose(1, 2, 0))
    wstrip_real = np.ascontiguousarray(w3[256]).astype(W_NP)      # (257, H)
    wstrip_zero = np.zeros((NP, H), W_NP)

    x1T = np.ascontiguousarray(x1.T)  # (256 j-1, 256 b)
    # j indices of the main loop, paired: jj[c, t, par] = 1 + 16c + 2t + par
    jj = np.arange(1, 257).reshape(NCHUNK, JC, 2)

    in_maps = []
    for core in range(8):
        bp, kq = core // P_K, core % P_K
        i0 = kq * IQ
        xs = np.ascontiguousarray(x1T[:, bp * BC : (bp + 1) * BC])  # (256, BC)
        a0T = np.concatenate([np.zeros((1, BC), np.float32), xs])
        a1T = np.concatenate([np.ones((1, BC), np.float32), xs])
        # aflat[2c+par, t*BC+b] = xs[jj[c,t,par]-1, b]
        af = xs[jj - 1, :]                     # (NCHUNK, JC, 2, BC)
        aflat = np.ascontiguousarray(
            af.transpose(0, 2, 1, 3).reshape(2 * NCHUNK, CH)).astype(W_NP)

        wi = w3[i0 : i0 + IQ]                  # (64, 257, H)
        wq = wi[:, jj, :]                      # (64, NCHUNK, JC, 2, H)
        wq = np.ascontiguousarray(
            wq.transpose(3, 0, 1, 2, 4).reshape(128, NCHUNK, JC * H)
        ).astype(W_NP)

        in_maps.append({
            "a0T": a0T, "a1T": a1T, "aflat": aflat,
            "b0": np.tile(b0[i0 : i0 + IQ], 2).reshape(128, 1).copy(),
            "b1": np.tile(b1[i0 : i0 + IQ], 2).reshape(128, 1).copy(),
            "cv": cv, "sv": sv, "ind": ind,
            "yj0": np.ascontiguousarray(y0[i0 : i0 + IQ]).reshape(IQ, 1),
            "w3": wq,
            "wcol0": np.ascontiguousarray(wi[:, 0, :]).astype(W_NP),
            "wstrip": wstrip_real if kq == P_K - 1 else wstrip_zero,
        })
    return in_maps


def kernel(**inputs):
    in_maps = make_in_maps(**inputs)
    nc = _get_program()
    res = run_bass_kernel_spmd(nc, in_maps, list(range(8)))

    fc_b = np.asarray(inputs["fc_b"], np.float32)
    out_w = np.asarray(inputs["out_w"], np.float32)
    out_b = np.asarray(inputs["out_b"], np.float32)

    out = np.zeros((B, C), np.float32)
    for bp in range(P_B):
        y2 = np.zeros((BC, H), np.float32)
        for kq in range(P_K):
            y2 += res.results[bp * P_K + kq]["out"]
        y2 = np.maximum(y2 + fc_b, 0.0)
        out[bp * BC : (bp + 1) * BC] = y2 @ out_w.T + out_b
    return out
